# revision 24
# baseline (speedup 1.0000x reference)
"""AFT-Full attention kernel for 8 TRN2 NeuronCores.

Reference computation (S=2048, B=16, D=512):
    q = query @ Wq.T + bq
    k = key @ Wk.T + bk
    v = k @ Wv.T + bv
    num = exp_pb @ (exp(k) * v);  den = exp_pb @ exp(k)   (per batch)
    out = (sigmoid(q) * num / den).transpose(1,0,2) @ Wo.T + bo

Sharding: data-parallel over batch B: 2 batches per core, no collectives.
Math notes:
  - the max-subtractions in the reference cancel exactly in num/den; values are
    small enough that fp32 exp is safe without them.
  - v = k @ Wv.T = key @ (Wv @ Wk).T  -> v computed directly from key with a
    host-folded weight, so k and v share the same stationary operand.
  - bq/bk are absorbed into query/key on the host via inv(W.T); bo is added on
    the host after gather. (All biases are zero for this problem's inputs.)
  - exp(pos_bias) is precomputed on the host (it is batch-independent).
Matmuls run in bf16 (1 cycle/row on TensorE; fp32 PSUM accumulation).
All DRAM inputs are pre-tiled on the host partition-major so every DMA moves
>=4 KB contiguous per partition. The finalize (gating + output projection) of
i-tile T is emitted after the einsum of i-tile T+1 so the TensorEngine never
idles waiting for the vector-engine epilogue.
"""
import sys

sys.path.insert(0, "/opt/trn_rl_repo")

import numpy as np

S, B, D = 2048, 16, 512
NCORES = 8
BLOC = B // NCORES          # 2 batches per core
ST = S // 128               # 16 seq (j) tiles
DT = D // 128               # 4 feature tiles
NI = S // 128               # 16 output (i) tiles
PBG = 4                     # pos-bias i-tiles per DMA group

_cache = {}


def _build(use_kv: bool):
    import concourse.bacc as bacc
    import concourse.mybir as mybir
    import concourse.tile as tile
    from concourse.masks import make_identity

    f32 = mybir.dt.float32
    bf16 = mybir.dt.bfloat16
    ACT = mybir.ActivationFunctionType

    nc = bacc.Bacc()

    # activations pre-tiled on host, partition-major: [b, p, st, kt, 128]
    # element [b, p, st, kt, sl] = x.T[kt*128+p, st*128+sl]  (x.T is [D, S])
    qT = nc.declare_dram_parameter("qT", [BLOC, 128, ST, DT, 128], bf16, isOutput=False)
    kT = nc.declare_dram_parameter("kT", [BLOC, 128, ST, DT, 128], bf16, isOutput=False)
    kTv = (
        nc.declare_dram_parameter("kTv", [BLOC, 128, ST, DT, 128], bf16, isOutput=False)
        if use_kv
        else kT
    )
    # exp(pos_bias).T pre-tiled partition-major: [p, it, jt, 128i]
    pbt = nc.declare_dram_parameter("pbt", [128, NI, ST, 128], bf16, isOutput=False)
    # weights pre-tiled: [p, kt, dout] with din = kt*128+p
    wk = nc.declare_dram_parameter("wk", [128, DT, D], bf16, isOutput=False)
    wvk = nc.declare_dram_parameter("wvk", [128, DT, D], bf16, isOutput=False)
    wq = nc.declare_dram_parameter("wq", [128, DT, D], bf16, isOutput=False)
    wo = nc.declare_dram_parameter("wo", [128, DT, D], bf16, isOutput=False)
    out = nc.declare_dram_parameter("out", [BLOC, S, D], f32, isOutput=True)

    with tile.TileContext(nc) as tc:
        with (
            tc.tile_pool(name="big", bufs=1) as big,
            tc.tile_pool(name="psum", bufs=1, space="PSUM") as psum,
        ):
            # persistent exp(k), exp(k)*v per local batch: [p, jt, d], j = jt*128+p
            E = [big.tile([128, ST, D], bf16, name=f"E{b}") for b in range(BLOC)]
            Ev = [big.tile([128, ST, D], bf16, name=f"Ev{b}") for b in range(BLOC)]
            ident = big.tile([128, 128], bf16, name="ident")
            make_identity(nc, ident)

            # PE warmup: keep TensorE busy while the first DMAs stream so the
            # HAM clock-gate opens before the first real matmul
            ps_warm = psum.tile([128, 128], f32, tag="warm")
            for _ in range(48):
                nc.tensor.matmul(ps_warm[:, :], ident[:, :], ident[:, :])

            # ---------------- phase 1: projections k, v -> E, Ev ------------
            with (
                tc.tile_pool(name="ph1", bufs=1) as ph1,
                tc.tile_pool(name="ph1s", bufs=2) as ph1s,
            ):
                wk_sb = ph1.tile([128, DT, D], bf16)
                nc.sync.dma_start(wk_sb[:, :, :], wk[:, :, :])
                wvk_sb = ph1.tile([128, DT, D], bf16)
                nc.sync.dma_start(wvk_sb[:, :, :], wvk[:, :, :])

                # graduated chunks: small first so compute starts early
                CHUNKS = [(0, 1), (1, 2), (2, 4), (4, 8), (8, 16)]
                for b in range(BLOC):
                    # whole-batch kT resident, chunked DMAs
                    kfull = ph1s.tile([128, ST, DT, 128], bf16, tag="kfull")
                    for lo, hi in CHUNKS:
                        nc.sync.dma_start(kfull[:, lo:hi], kT[b, :, lo:hi])
                    if use_kv:
                        kvfull = ph1s.tile([128, ST, DT, 128], bf16, tag="kvfull")
                        for lo, hi in CHUNKS:
                            nc.sync.dma_start(kvfull[:, lo:hi], kTv[b, :, lo:hi])
                    else:
                        kvfull = kfull
                    for jt in range(ST):
                        ps_k = psum.tile([128, D], f32, tag="nd0")
                        for kt in range(DT):
                            nc.tensor.matmul(
                                ps_k[:, :],
                                kfull[:, jt, kt, :],
                                wk_sb[:, kt, :],
                                start=(kt == 0),
                                stop=(kt == DT - 1),
                            )
                        ps_v = psum.tile([128, D], f32, tag="nd2")
                        for kt in range(DT):
                            nc.tensor.matmul(
                                ps_v[:, :],
                                kvfull[:, jt, kt, :],
                                wvk_sb[:, kt, :],
                                start=(kt == 0),
                                stop=(kt == DT - 1),
                            )
                        nc.scalar.activation(E[b][:, jt, :], ps_k[:, :], ACT.Exp)
                        nc.vector.tensor_mul(
                            Ev[b][:, jt, :], E[b][:, jt, :], ps_v[:, :]
                        )

            # ---------------- phase 2: einsum + gating + output -------------
            with (
                tc.tile_pool(name="ph2", bufs=1) as ph2,
                tc.tile_pool(name="ph2pb", bufs=2) as ph2pb,
                tc.tile_pool(name="fin", bufs=3) as fin,
                tc.tile_pool(name="nds", bufs=3) as nds,
            ):
                pb_tiles = {}

                def ensure_pb_group(g):
                    if g in pb_tiles or g >= NI // PBG:
                        return
                    t = ph2pb.tile([128, PBG, ST, 128], bf16, tag="pb_sb")
                    for c in range(PBG):
                        nc.sync.dma_start(t[:, c], pbt[:, g * PBG + c])
                    pb_tiles[g] = t

                ensure_pb_group(0)

                wq_sb = ph2.tile([128, DT, D], bf16)
                nc.sync.dma_start(wq_sb[:, :, :], wq[:, :, :])
                wo_sb = ph2.tile([128, DT, D], bf16)
                nc.sync.dma_start(wo_sb[:, :, :], wo[:, :, :])
                # whole-batch qT resident (2 MB/batch), chunked DMAs
                qfull = []
                for b in range(BLOC):
                    t = ph2.tile([128, NI, DT, 128], bf16, name=f"qfull{b}")
                    for c in range(4):
                        csl = slice(c * (NI // 4), (c + 1) * (NI // 4))
                        nc.sync.dma_start(t[:, csl], qT[b, :, csl])
                    qfull.append(t)

                def einsum_step(it, evac=True):
                    pb_g = pb_tiles[it // PBG]
                    ps_nd = [
                        psum.tile([128, D], f32, tag=f"nd{x}", name=f"nd{x}_{it}")
                        for x in range(2 * BLOC)
                    ]
                    for jt in range(ST):
                        lhs = pb_g[:, it % PBG, jt, :]
                        for b in range(BLOC):
                            nc.tensor.matmul(
                                ps_nd[2 * b][:, :],
                                lhs,
                                Ev[b][:, jt, :],
                                start=(jt == 0),
                                stop=(jt == ST - 1),
                            )
                            nc.tensor.matmul(
                                ps_nd[2 * b + 1][:, :],
                                lhs,
                                E[b][:, jt, :],
                                start=(jt == 0),
                                stop=(jt == ST - 1),
                            )
                    if not evac:
                        return ps_nd
                    # evacuate to SBUF so the banks free up for the next i-tile
                    nd_sb = []
                    for x in range(2 * BLOC):
                        t = nds.tile([128, D], f32, tag=f"nds{x}", name=f"nds{x}_{it}")
                        nc.vector.tensor_copy(t[:, :], ps_nd[x][:, :])
                        nd_sb.append(t)
                    return nd_sb

                def finalize_step(it, nd_sb):
                    isl = slice(it * 128, (it + 1) * 128)
                    for b in range(BLOC):
                        ps_q = psum.tile([128, D], f32, tag="ps_q")
                        for kt in range(DT):
                            nc.tensor.matmul(
                                ps_q[:, :],
                                qfull[b][:, it, kt, :],
                                wq_sb[:, kt, :],
                                start=(kt == 0),
                                stop=(kt == DT - 1),
                            )
                        sig = fin.tile([128, D], f32, tag="sig")
                        nc.scalar.activation(sig[:, :], ps_q[:, :], ACT.Sigmoid)
                        rec = fin.tile([128, D], f32, tag="rec")
                        nc.vector.reciprocal_approx_fast(
                            rec[:, :], nd_sb[2 * b + 1][:, :]
                        )
                        w = fin.tile([128, D], f32, tag="w")
                        nc.vector.tensor_mul(w[:, :], nd_sb[2 * b][:, :], rec[:, :])
                        y = fin.tile([128, D], bf16, tag="y")
                        nc.vector.tensor_mul(y[:, :], w[:, :], sig[:, :])
                        # transpose y -> yT [dk, s] (4 blocks into one PSUM bank)
                        ps_t = psum.tile([128, D], bf16, tag="ps_t")
                        for kt in range(DT):
                            nc.tensor.transpose(
                                ps_t[:, kt * 128 : (kt + 1) * 128],
                                y[:, kt * 128 : (kt + 1) * 128],
                                ident,
                            )
                        yT = fin.tile([128, DT, 128], bf16, tag="yT")
                        nc.vector.tensor_copy(
                            yT[:, :, :],
                            ps_t[:, :].rearrange("p (kt s) -> p kt s", kt=DT),
                        )
                        ps_o = psum.tile([128, D], f32, tag="ps_o")
                        for kt in range(DT):
                            nc.tensor.matmul(
                                ps_o[:, :],
                                yT[:, kt, :],
                                wo_sb[:, kt, :],
                                start=(kt == 0),
                                stop=(kt == DT - 1),
                            )
                        o_sb = fin.tile([128, D], f32, tag="o_sb")
                        nc.scalar.copy(o_sb[:, :], ps_o[:, :])
                        nc.sync.dma_start(out[b, isl, :], o_sb[:, :])

                # software pipeline: finalize(it-1) emitted after einsum(it)
                prev = None
                for it in range(NI):
                    if it % PBG == 1:
                        ensure_pb_group(it // PBG + 1)
                    # last i-tile: no bank reuse follows, so finalize reads
                    # num/den straight from PSUM (keeps the tail short)
                    nd_sb = einsum_step(it, evac=(it < NI - 1))
                    if prev is not None:
                        finalize_step(it - 1, prev)
                    prev = nd_sb
                finalize_step(NI - 1, prev)

    nc.compile()
    return nc


def _tile_act(xT):
    """[D, S] -> [p, st, kt, 128] partition-major host tiling."""
    # xT[kt*128+p, st*128+sl] -> Z[p, st, kt, sl]
    z = xT.reshape(DT, 128, ST, 128)
    return np.ascontiguousarray(z.transpose(1, 2, 0, 3))


def _tile_w(wT):
    """[D, D] (din, dout) -> [p, kt, dout] with din = kt*128+p."""
    return np.ascontiguousarray(wT.reshape(DT, 128, D).transpose(1, 0, 2))


def _prep(query, key, Wq, bq, Wk, bk, Wv, bv, pos_bias, Wo, bo):
    """Host-side preprocessing: transposes, tiling, bias absorption, bf16."""
    import ml_dtypes

    bf16 = ml_dtypes.bfloat16

    query = np.asarray(query, dtype=np.float32)
    key = np.asarray(key, dtype=np.float32)
    Wq = np.asarray(Wq, dtype=np.float32)
    Wk = np.asarray(Wk, dtype=np.float32)
    Wv = np.asarray(Wv, dtype=np.float32)
    Wo = np.asarray(Wo, dtype=np.float32)
    bq = np.asarray(bq, dtype=np.float32)
    bk = np.asarray(bk, dtype=np.float32)
    bv = np.asarray(bv, dtype=np.float32)
    bo = np.asarray(bo, dtype=np.float32)

    Wvk = Wv @ Wk

    if np.any(bq):
        query = query + np.linalg.solve(Wq, bq).astype(np.float32)
    if np.any(bk):
        key_k = key + np.linalg.solve(Wk, bk).astype(np.float32)
    else:
        key_k = key
    use_kv = bool(np.any(bv)) or bool(np.any(bk))
    if use_kv:
        bv_eff = Wv @ bk + bv
        key_v = key + np.linalg.solve(Wvk, bv_eff).astype(np.float32)
    else:
        key_v = None

    # [S, B, D] -> per-batch [D, S] -> tiled [B, 128, ST, DT, 128] bf16
    qTb = query.transpose(1, 2, 0).astype(bf16)
    kTb = key_k.transpose(1, 2, 0).astype(bf16)
    qT = np.stack([_tile_act(qTb[b]) for b in range(B)])
    kT = np.stack([_tile_act(kTb[b]) for b in range(B)])
    if use_kv:
        kvb = key_v.transpose(1, 2, 0).astype(bf16)
        kTv = np.stack([_tile_act(kvb[b]) for b in range(B)])
    else:
        kTv = None

    # exp(pos_bias).T tiled partition-major: [p, it, jt, 128]
    expPbT = np.exp(np.asarray(pos_bias, dtype=np.float32)).T.astype(bf16)
    pbt = np.ascontiguousarray(
        expPbT.reshape(ST, 128, NI, 128).transpose(1, 2, 0, 3)
    )

    wk = _tile_w(np.ascontiguousarray(Wk.T).astype(bf16))
    wvk = _tile_w(np.ascontiguousarray(Wvk.T).astype(bf16))
    wq = _tile_w(np.ascontiguousarray(Wq.T).astype(bf16))
    wo = _tile_w(np.ascontiguousarray(Wo.T).astype(bf16))
    return qT, kT, kTv, pbt, wk, wvk, wq, wo, bo, use_kv


def kernel(query, key, Wq, bq, Wk, bk, Wv, bv, pos_bias, Wo, bo):
    from concourse.bass_utils import run_bass_kernel_spmd

    qT, kT, kTv, pbt, wk, wvk, wq, wo, bo, use_kv = _prep(
        query, key, Wq, bq, Wk, bk, Wv, bv, pos_bias, Wo, bo
    )

    if ("nc", use_kv) not in _cache:
        _cache[("nc", use_kv)] = _build(use_kv)
    nc = _cache[("nc", use_kv)]

    in_maps = []
    for c in range(NCORES):
        bsl = slice(c * BLOC, (c + 1) * BLOC)
        m = {
            "qT": qT[bsl],
            "kT": kT[bsl],
            "pbt": pbt,
            "wk": wk,
            "wvk": wvk,
            "wq": wq,
            "wo": wo,
        }
        if use_kv:
            m["kTv"] = kTv[bsl]
        in_maps.append(m)

    res = run_bass_kernel_spmd(nc, in_maps, core_ids=list(range(NCORES)))
    out = np.concatenate([res.results[c]["out"] for c in range(NCORES)], axis=0)
    if np.any(bo):
        out = out + bo
    return out


# revision 29
# speedup vs baseline: 1.0272x; 1.0272x over previous
"""AFT-Full attention kernel for 8 TRN2 NeuronCores.

Reference computation (S=2048, B=16, D=512):
    q = query @ Wq.T + bq
    k = key @ Wk.T + bk
    v = k @ Wv.T + bv
    num = exp_pb @ (exp(k) * v);  den = exp_pb @ exp(k)   (per batch)
    out = (sigmoid(q) * num / den).transpose(1,0,2) @ Wo.T + bo

Sharding: data-parallel over batch B: 2 batches per core, no collectives.
Math notes:
  - the max-subtractions in the reference cancel exactly in num/den; values are
    small enough that fp32 exp is safe without them.
  - v = k @ Wv.T = key @ (Wv @ Wk).T  -> v computed directly from key with a
    host-folded weight, so k and v share the same stationary operand.
  - bq/bk are absorbed into query/key on the host via inv(W.T); bo is added on
    the host after gather. (All biases are zero for this problem's inputs.)
  - exp(pos_bias) is precomputed on the host (it is batch-independent).
Matmuls run in bf16 (1 cycle/row on TensorE; fp32 PSUM accumulation).
All DRAM inputs are pre-tiled on the host partition-major so every DMA moves
>=4 KB contiguous per partition.

The einsum computes num/den TRANSPOSED (numT[d, i] = sum_j Ev[j, d] pbT[j, i],
with the exp(k)*v tiles stationary and exp(pos_bias).T moving), so the gating
and the output projection all run in [d, s] layout and no PE transposes are
needed anywhere. The gating of chunk T is emitted after the einsum of chunk
T+1 so the TensorEngine never idles waiting for the vector-engine epilogue.
"""
import sys

sys.path.insert(0, "/opt/trn_rl_repo")

import numpy as np

S, B, D = 2048, 16, 512
NCORES = 8
BLOC = B // NCORES          # 2 batches per core
ST = S // 128               # 16 seq (j) tiles
DT = D // 128               # 4 feature tiles
NI = S // 128               # 16 output (i) tiles
NC = S // 512               # 4 output column-chunks (512 wide)
PBG = NI // NC              # pos-bias i-tiles per DMA group (= one n-chunk)

_cache = {}


def _build(use_kv: bool):
    import concourse.bacc as bacc
    import concourse.mybir as mybir
    import concourse.tile as tile

    f32 = mybir.dt.float32
    bf16 = mybir.dt.bfloat16
    ACT = mybir.ActivationFunctionType

    nc = bacc.Bacc()

    # key pre-tiled partition-major: [b, p, st, kt, 128] (lhsT tiles for the
    # k/v projections); element = key.T[kt*128+p, st*128+sl] per batch
    kT = nc.declare_dram_parameter("kT", [BLOC, 128, ST, DT, 128], bf16, isOutput=False)
    kTv = (
        nc.declare_dram_parameter("kTv", [BLOC, 128, ST, DT, 128], bf16, isOutput=False)
        if use_kv
        else kT
    )
    # query pre-tiled as moving tiles: [b, p, kt, s]; element = q.T[kt*128+p, s]
    qT = nc.declare_dram_parameter("qT", [BLOC, 128, DT, S], bf16, isOutput=False)
    # exp(pos_bias).T pre-tiled partition-major: [p, it, jt, 128i]
    pbt = nc.declare_dram_parameter("pbt", [128, NI, ST, 128], bf16, isOutput=False)
    # weights pre-tiled: [p, kt, dout] with din = kt*128+p
    wk = nc.declare_dram_parameter("wk", [128, DT, D], bf16, isOutput=False)
    wvk = nc.declare_dram_parameter("wvk", [128, DT, D], bf16, isOutput=False)
    wq = nc.declare_dram_parameter("wq", [128, DT, D], bf16, isOutput=False)
    wo = nc.declare_dram_parameter("wo", [128, DT, D], bf16, isOutput=False)
    out = nc.declare_dram_parameter("out", [BLOC, S, D], f32, isOutput=True)

    with tile.TileContext(nc) as tc:
        with (
            tc.tile_pool(name="big", bufs=1) as big,
            tc.tile_pool(name="psum", bufs=1, space="PSUM") as psum,
        ):
            # persistent exp(k), exp(k)*v per local batch: [p, jt, d], j = jt*128+p
            E = [big.tile([128, ST, D], bf16, name=f"E{b}") for b in range(BLOC)]
            Ev = [big.tile([128, ST, D], bf16, name=f"Ev{b}") for b in range(BLOC)]
            warm_src = big.tile([128, 128], bf16, name="warm_src")
            nc.vector.memset(warm_src[:, :], 1.0)

            # PE warmup: keep TensorE busy while the first DMAs stream so the
            # HAM clock-gate opens before the first real matmul
            ps_warm = psum.tile([128, 128], f32, tag="ps_q", bufs=2)
            for _ in range(48):
                nc.tensor.matmul(ps_warm[:, :], warm_src[:, :], warm_src[:, :])

            # ---------------- phase 1: projections k, v -> E, Ev ------------
            with (
                tc.tile_pool(name="ph1", bufs=1) as ph1,
                tc.tile_pool(name="ph1s", bufs=2) as ph1s,
            ):
                wk_sb = ph1.tile([128, DT, D], bf16)
                nc.sync.dma_start(wk_sb[:, :, :], wk[:, :, :])
                wvk_sb = ph1.tile([128, DT, D], bf16)
                nc.sync.dma_start(wvk_sb[:, :, :], wvk[:, :, :])

                # graduated chunks: small first so compute starts early
                CHUNKS = [(0, 1), (1, 2), (2, 4), (4, 8), (8, 16)]
                for b in range(BLOC):
                    kfull = ph1s.tile([128, ST, DT, 128], bf16, tag="kfull")
                    for lo, hi in CHUNKS:
                        nc.sync.dma_start(kfull[:, lo:hi], kT[b, :, lo:hi])
                    if use_kv:
                        kvfull = ph1s.tile([128, ST, DT, 128], bf16, tag="kvfull")
                        for lo, hi in CHUNKS:
                            nc.sync.dma_start(kvfull[:, lo:hi], kTv[b, :, lo:hi])
                    else:
                        kvfull = kfull
                    for jt in range(ST):
                        ps_k = psum.tile([128, D], f32, tag="nd0")
                        for kt in range(DT):
                            nc.tensor.matmul(
                                ps_k[:, :],
                                kfull[:, jt, kt, :],
                                wk_sb[:, kt, :],
                                start=(kt == 0),
                                stop=(kt == DT - 1),
                            )
                        ps_v = psum.tile([128, D], f32, tag="nd2")
                        for kt in range(DT):
                            nc.tensor.matmul(
                                ps_v[:, :],
                                kvfull[:, jt, kt, :],
                                wvk_sb[:, kt, :],
                                start=(kt == 0),
                                stop=(kt == DT - 1),
                            )
                        nc.scalar.activation(E[b][:, jt, :], ps_k[:, :], ACT.Exp)
                        nc.vector.tensor_mul(
                            Ev[b][:, jt, :], E[b][:, jt, :], ps_v[:, :]
                        )

            # ------- phase 2: transposed einsum + gating + output -----------
            with (
                tc.tile_pool(name="ph2", bufs=1) as ph2,
                tc.tile_pool(name="ph2pb", bufs=2) as ph2pb,
                tc.tile_pool(name="fin", bufs=3) as fin,
                tc.tile_pool(name="nds", bufs=3) as nds,
                tc.tile_pool(name="yts", bufs=2) as yts,
            ):
                pb_tiles = {}

                def ensure_pb_group(g):
                    if g in pb_tiles or g >= NC:
                        return
                    t = ph2pb.tile([128, PBG, ST, 128], bf16, tag="pb_sb")
                    for c in range(PBG):
                        nc.sync.dma_start(t[:, c], pbt[:, g * PBG + c])
                    pb_tiles[g] = t

                ensure_pb_group(0)

                wq_sb = ph2.tile([128, DT, D], bf16)
                nc.sync.dma_start(wq_sb[:, :, :], wq[:, :, :])
                wo_sb = ph2.tile([128, DT, D], bf16)
                nc.sync.dma_start(wo_sb[:, :, :], wo[:, :, :])
                # whole-batch qT resident (2 MB/batch), chunked DMAs
                qfull = []
                for b in range(BLOC):
                    t = ph2.tile([128, DT, S], bf16, name=f"qfull{b}")
                    for c in range(4):
                        csl = slice(c * (S // 4), (c + 1) * (S // 4))
                        nc.sync.dma_start(t[:, :, csl], qT[b, :, :, csl])
                    qfull.append(t)

                def einsum_step(n, m, evac=True):
                    # numT/denT [d-chunk 128, i-chunk 512] for both batches
                    pb_g = pb_tiles[n]
                    msl = slice(m * 128, (m + 1) * 128)
                    ps_nd = [
                        psum.tile([128, 512], f32, tag=f"nd{x}", name=f"nd{x}_{n}_{m}")
                        for x in range(2 * BLOC)
                    ]
                    for jt in range(ST):
                        rhs = pb_g[:, :, jt, :]  # [128, PBG, 128] = 512 free
                        for b in range(BLOC):
                            nc.tensor.matmul(
                                ps_nd[2 * b][:, :],
                                Ev[b][:, jt, msl],
                                rhs,
                                start=(jt == 0),
                                stop=(jt == ST - 1),
                            )
                            nc.tensor.matmul(
                                ps_nd[2 * b + 1][:, :],
                                E[b][:, jt, msl],
                                rhs,
                                start=(jt == 0),
                                stop=(jt == ST - 1),
                            )
                    if not evac:
                        return ps_nd
                    # evacuate so banks free up for the next (n, m)
                    nd_sb = []
                    for x in range(2 * BLOC):
                        t = nds.tile(
                            [128, 512], f32, tag=f"nds{x}", name=f"nds{x}_{n}_{m}"
                        )
                        nc.vector.tensor_copy(t[:, :], ps_nd[x][:, :])
                        nd_sb.append(t)
                    return nd_sb

                def gate_step(n, m, nd_sb, yT):
                    # qT chunk, sigmoid, gating -> yT[b][:, m, :] ([d, s])
                    nsl = slice(n * 512, (n + 1) * 512)
                    msl = slice(m * 128, (m + 1) * 128)
                    for b in range(BLOC):
                        ps_q = psum.tile([128, 512], f32, tag="ps_q", bufs=2)
                        for kt in range(DT):
                            nc.tensor.matmul(
                                ps_q[:, :],
                                wq_sb[:, kt, msl],
                                qfull[b][:, kt, nsl],
                                start=(kt == 0),
                                stop=(kt == DT - 1),
                            )
                        sig = fin.tile([128, 512], f32, tag="sig")
                        nc.scalar.activation(sig[:, :], ps_q[:, :], ACT.Sigmoid)
                        rec = fin.tile([128, 512], f32, tag="rec")
                        nc.vector.reciprocal_approx_fast(
                            rec[:, :], nd_sb[b * 2 + 1][:, :]
                        )
                        w = fin.tile([128, 512], f32, tag="w")
                        nc.vector.tensor_mul(w[:, :], nd_sb[b * 2][:, :], rec[:, :])
                        nc.vector.tensor_mul(yT[b][:, m, :], w[:, :], sig[:, :])

                def output_step(n, yT):
                    # out[s, dout] for the 4 s-subtiles of this n-chunk
                    for b in range(BLOC):
                        for ssub in range(4):
                            s0 = n * 512 + ssub * 128
                            ps_o = psum.tile([128, D], f32, tag="ps_o", bufs=2)
                            for dk in range(DT):
                                nc.tensor.matmul(
                                    ps_o[:, :],
                                    yT[b][:, dk, ssub * 128 : (ssub + 1) * 128],
                                    wo_sb[:, dk, :],
                                    start=(dk == 0),
                                    stop=(dk == DT - 1),
                                )
                            o_sb = fin.tile([128, D], f32, tag="o_sb")
                            nc.scalar.copy(o_sb[:, :], ps_o[:, :])
                            nc.sync.dma_start(out[b, s0 : s0 + 128, :], o_sb[:, :])

                # software pipeline over (n, m) chunks: gate(prev) after
                # einsum(cur); output projection once an n-chunk's yT is full
                prev = None          # (n, m, nd_sb)
                yT_tiles = {}
                for n in range(NC):
                    yT_tiles[n] = [
                        yts.tile([128, DT, 512], bf16, tag=f"yT{b}", name=f"yT{b}_{n}")
                        for b in range(BLOC)
                    ]
                    if n + 1 < NC:
                        ensure_pb_group(n + 1)
                    for m in range(DT):
                        last = n == NC - 1 and m == DT - 1
                        nd_sb = einsum_step(n, m, evac=not last)
                        if prev is not None:
                            pn, pm, pnd = prev
                            gate_step(pn, pm, pnd, yT_tiles[pn])
                            if pm == DT - 1:
                                output_step(pn, yT_tiles[pn])
                                del yT_tiles[pn]
                        prev = (n, m, nd_sb)
                pn, pm, pnd = prev
                gate_step(pn, pm, pnd, yT_tiles[pn])
                output_step(pn, yT_tiles[pn])

    nc.compile()
    return nc


def _tile_act(xT):
    """[D, S] -> [p, st, kt, 128] partition-major host tiling (lhsT tiles)."""
    z = xT.reshape(DT, 128, ST, 128)
    return np.ascontiguousarray(z.transpose(1, 2, 0, 3))


def _tile_mov(xT):
    """[D, S] -> [p, kt, S] partition-major host tiling (moving tiles)."""
    z = xT.reshape(DT, 128, S)
    return np.ascontiguousarray(z.transpose(1, 0, 2))


def _tile_w(wT):
    """[D, D] (din, dout) -> [p, kt, dout] with din = kt*128+p."""
    return np.ascontiguousarray(wT.reshape(DT, 128, D).transpose(1, 0, 2))


def _prep(query, key, Wq, bq, Wk, bk, Wv, bv, pos_bias, Wo, bo):
    """Host-side preprocessing: transposes, tiling, bias absorption, bf16."""
    import ml_dtypes

    bf16 = ml_dtypes.bfloat16

    query = np.asarray(query, dtype=np.float32)
    key = np.asarray(key, dtype=np.float32)
    Wq = np.asarray(Wq, dtype=np.float32)
    Wk = np.asarray(Wk, dtype=np.float32)
    Wv = np.asarray(Wv, dtype=np.float32)
    Wo = np.asarray(Wo, dtype=np.float32)
    bq = np.asarray(bq, dtype=np.float32)
    bk = np.asarray(bk, dtype=np.float32)
    bv = np.asarray(bv, dtype=np.float32)
    bo = np.asarray(bo, dtype=np.float32)

    Wvk = Wv @ Wk

    if np.any(bq):
        query = query + np.linalg.solve(Wq, bq).astype(np.float32)
    if np.any(bk):
        key_k = key + np.linalg.solve(Wk, bk).astype(np.float32)
    else:
        key_k = key
    use_kv = bool(np.any(bv)) or bool(np.any(bk))
    if use_kv:
        bv_eff = Wv @ bk + bv
        key_v = key + np.linalg.solve(Wvk, bv_eff).astype(np.float32)
    else:
        key_v = None

    # [S, B, D] -> per-batch [D, S] -> tiled bf16
    qTb = query.transpose(1, 2, 0).astype(bf16)
    kTb = key_k.transpose(1, 2, 0).astype(bf16)
    qT = np.stack([_tile_mov(qTb[b]) for b in range(B)])
    kT = np.stack([_tile_act(kTb[b]) for b in range(B)])
    if use_kv:
        kvb = key_v.transpose(1, 2, 0).astype(bf16)
        kTv = np.stack([_tile_act(kvb[b]) for b in range(B)])
    else:
        kTv = None

    # exp(pos_bias).T tiled partition-major: [p, it, jt, 128]
    expPbT = np.exp(np.asarray(pos_bias, dtype=np.float32)).T.astype(bf16)
    pbt = np.ascontiguousarray(
        expPbT.reshape(ST, 128, NI, 128).transpose(1, 2, 0, 3)
    )

    wk = _tile_w(np.ascontiguousarray(Wk.T).astype(bf16))
    wvk = _tile_w(np.ascontiguousarray(Wvk.T).astype(bf16))
    wq = _tile_w(np.ascontiguousarray(Wq.T).astype(bf16))
    wo = _tile_w(np.ascontiguousarray(Wo.T).astype(bf16))
    return qT, kT, kTv, pbt, wk, wvk, wq, wo, bo, use_kv


def kernel(query, key, Wq, bq, Wk, bk, Wv, bv, pos_bias, Wo, bo):
    from concourse.bass_utils import run_bass_kernel_spmd

    qT, kT, kTv, pbt, wk, wvk, wq, wo, bo, use_kv = _prep(
        query, key, Wq, bq, Wk, bk, Wv, bv, pos_bias, Wo, bo
    )

    if ("nc", use_kv) not in _cache:
        _cache[("nc", use_kv)] = _build(use_kv)
    nc = _cache[("nc", use_kv)]

    in_maps = []
    for c in range(NCORES):
        bsl = slice(c * BLOC, (c + 1) * BLOC)
        m = {
            "qT": qT[bsl],
            "kT": kT[bsl],
            "pbt": pbt,
            "wk": wk,
            "wvk": wvk,
            "wq": wq,
            "wo": wo,
        }
        if use_kv:
            m["kTv"] = kTv[bsl]
        in_maps.append(m)

    res = run_bass_kernel_spmd(nc, in_maps, core_ids=list(range(NCORES)))
    out = np.concatenate([res.results[c]["out"] for c in range(NCORES)], axis=0)
    if np.any(bo):
        out = out + bo
    return out
